# revision 10
# baseline (speedup 1.0000x reference)
"""Trainium2 Bass kernel for nn_CAPAtt: embed+LSTM cell -> additive attention
-> gated caption cell -> vocab projection, distributed over 8 NeuronCores.

Sharding:
  - LSTM / caption cells: H-sharded (each core computes a 128-wide slice of
    the 1024 hidden features for all 256 batches), weights column-sharded.
  - Attention: batch-sharded (each core handles 32 batches, attention weights
    replicated).
  - MLP vocab projection: vocab-sharded (4000 vocab rows per core).
  - Three AllGathers stitch the stages: h1, alpha_v, h2 (all tiny, ~1MB).

All device tensors that feed matmuls live in "transposed" layout with the
contraction dim on partitions; host pre-transposes weights and initial
states (free), and final outputs are re-transposed on host.
"""

import numpy as np
import ml_dtypes

import concourse.bass as bass
import concourse.mybir as mybir
import concourse.tile as tile
from concourse import bacc
from concourse.bass_utils import run_bass_kernel_spmd
from concourse.masks import make_identity

F32 = mybir.dt.float32
BF16 = mybir.dt.bfloat16
U32 = mybir.dt.uint32
AF = mybir.ActivationFunctionType
ALU = mybir.AluOpType
AX = mybir.AxisListType

# Problem shape (hardcoded per contract)
B, N, H, E, V = 256, 36, 1024, 1024, 32000
NCORE = 8
P = 128
HS = H // NCORE          # 128  hidden slice per core
BS = B // NCORE          # 32   batch slice per core
VS = V // NCORE          # 4000 vocab slice per core
VSP = 4096               # padded vocab slice
KT = H // P              # 8    k-chunks of 128
BN = BS * N              # 1152 batch*spatial for attention
NCH = 4                  # attention bn chunks
BNCH = BN // NCH         # 288  (8 batches x 36 positions)
BCH = BS // NCH          # 8

MM_MODE = "bf16"         # "bf16" or "f32"


def _mm_dt(mode):
    return BF16 if mode == "bf16" else F32


def _host_dt(mode):
    return ml_dtypes.bfloat16 if mode == "bf16" else np.float32


def build_kernel(mode=MM_MODE):
    mdt = _mm_dt(mode)
    bf = mode == "bf16"
    MLP_CH = 512 if bf else 256            # vocab cols per mlp weight chunk
    N_MLP_CH = VSP // MLP_CH
    WBUFS = 8 if bf else 5
    MLPBUFS = 3 if bf else 3

    nc = bacc.Bacc(None, num_devices=NCORE)

    def inp(name, shape, dtype=mdt):
        return nc.dram_tensor(name, shape, dtype, kind="ExternalInput")

    def outp(name, shape):
        return nc.dram_tensor(name, shape, F32, kind="ExternalOutput")

    ids_d = inp("ids", [B], U32)
    h1sel_d = inp("h1sel", [H], U32)
    embed_d = inp("embed", [V, E])
    wihT_d = inp("wihT", [E, 4 * HS])
    whhT_d = inp("whhT", [H, 4 * HS])
    lstmb_d = inp("lstmb", [4 * HS], F32)
    h10T_d = inp("h10T", [H, B])
    c10Ts_d = inp("c10Ts", [HS, B], F32)
    vT_d = inp("vT", [H, BN])
    wvT_d = inp("wvT", [H, H])
    wgT_d = inp("wgT", [H, H])
    wcT_d = inp("wcT", [H, H])
    attb_d = inp("attb", [H], F32)
    whT_d = inp("whT", [H])
    c20Tb_d = inp("c20Tb", [H, BS])
    i2hT_d = inp("i2hT", [H, 4 * HS])
    h2hT_d = inp("h2hT", [H, 4 * HS])
    t2hT_d = inp("t2hT", [H, 2 * HS])
    capb_d = inp("capb", [4 * HS], F32)
    h20T_d = inp("h20T", [H, B])
    c20Ts_d = inp("c20Ts", [HS, B], F32)
    mlpT_d = inp("mlpT", [H, VSP])
    mlpb_d = inp("mlpb", [VSP])
    out_d = outp("logits", [B, VSP])
    h1s_d = outp("h1s", [HS, B])
    c1s_d = outp("c1s", [HS, B])
    h2s_d = outp("h2s", [HS, B])
    c2s_d = outp("c2s", [HS, B])

    def wstripe(dram_ap):
        # [K, M] weight panel -> SBUF [128, K//128, M]
        return dram_ap.rearrange("(kt p) m -> p kt m", p=P)

    def load(dst, src):
        # dtype-preserving loads on HWDGE; casts go through SWDGE
        if dst.dtype != src.dtype:
            nc.gpsimd.dma_start(dst, src)
        else:
            nc.sync.dma_start(dst, src)

    with tile.TileContext(nc) as tc:
        with (
            tc.tile_pool(name="const", bufs=1) as cpool,
            tc.tile_pool(name="acts", bufs=1) as apool,
            tc.tile_pool(name="wstream", bufs=WBUFS) as wpool,
            tc.tile_pool(name="mlpw", bufs=MLPBUFS) as mpool,
            tc.tile_pool(name="work", bufs=3) as tpool,
            tc.tile_pool(name="outs", bufs=3) as opool,
            tc.tile_pool(name="psum", bufs=7, space="PSUM") as ppool,
            tc.tile_pool(name="dram", bufs=1, space="DRAM") as dpool,
        ):
            def psum_t(free=512, parts=P, name="ps", dtype=F32):
                t = ppool.tile([P, 512], dtype, tag="ps", name=name)
                return t[:parts, :free]

            ident = cpool.tile([P, P], mdt)
            make_identity(nc, ident[:])

            # ---------------- stage A: embed gather + transpose ----------
            ids_sb = cpool.tile([P, 2], U32)
            x1T = apool.tile([P, KT, B], mdt)
            for t in range(2):
                nc.sync.dma_start(ids_sb[:, t : t + 1], ids_d[t * P : (t + 1) * P, None])
                xg = tpool.tile([P, E], mdt, tag="xg", name="xg")
                nc.gpsimd.indirect_dma_start(
                    out=xg[:],
                    out_offset=None,
                    in_=embed_d[:],
                    in_offset=bass.IndirectOffsetOnAxis(ap=ids_sb[:, t : t + 1], axis=0),
                )
                for kt in range(KT):
                    pt = psum_t(P, name="ps_tr", dtype=mdt)
                    nc.tensor.transpose(pt, xg[:, kt * P : (kt + 1) * P], ident[:])
                    nc.scalar.activation(
                        x1T[:, kt, t * P : (t + 1) * P], pt, AF.Relu
                    )

            # ---------------- stage B: LSTM cell (H-sharded) -------------
            h10T = apool.tile([P, KT, B], mdt)
            load(h10T[:], wstripe(h10T_d[:]))
            lstmb = cpool.tile([P, 4], F32)
            nc.sync.dma_start(lstmb[:], lstmb_d[:].rearrange("(g p) -> p g", p=P))
            c10Ts = apool.tile([P, B], F32)
            nc.sync.dma_start(c10Ts[:], c10Ts_d[:])

            gact = []
            # gate order: i, f, g, o
            for gt in range(4):
                wih = wpool.tile([P, KT, P], mdt, tag="w", name="wih")
                load(wih[:], wstripe(wihT_d[:, gt * P : (gt + 1) * P]))
                whh = wpool.tile([P, KT, P], mdt, tag="w", name="whh")
                load(whh[:], wstripe(whhT_d[:, gt * P : (gt + 1) * P]))
                pg = psum_t(B, name="ps_lstm")
                for kt in range(KT):
                    nc.tensor.matmul(
                        pg, lhsT=wih[:, kt, :], rhs=x1T[:, kt, :],
                        start=(kt == 0), stop=False,
                    )
                for kt in range(KT):
                    nc.tensor.matmul(
                        pg, lhsT=whh[:, kt, :], rhs=h10T[:, kt, :],
                        start=False, stop=(kt == KT - 1),
                    )
                ga = apool.tile([P, B], F32, name=f"gact{gt}")
                fn = AF.Tanh if gt == 2 else AF.Sigmoid
                nc.scalar.activation(ga[:], pg, fn, bias=lstmb[:, gt : gt + 1])
                gact.append(ga)

            c1s = apool.tile([P, B], F32)
            tmp_fc = tpool.tile([P, B], F32, tag="ew", name="tmp_fc")
            nc.vector.tensor_tensor(out=tmp_fc[:], in0=gact[1][:], in1=c10Ts[:], op=ALU.mult)
            tmp_ig = tpool.tile([P, B], F32, tag="ew", name="tmp_ig")
            nc.vector.tensor_tensor(out=tmp_ig[:], in0=gact[0][:], in1=gact[2][:], op=ALU.mult)
            nc.vector.tensor_tensor(out=c1s[:], in0=tmp_fc[:], in1=tmp_ig[:], op=ALU.add)
            tc1 = tpool.tile([P, B], F32, tag="ew", name="tc1")
            nc.scalar.activation(tc1[:], c1s[:], AF.Tanh)
            h1s = apool.tile([P, B], F32)
            nc.vector.tensor_tensor(out=h1s[:], in0=gact[3][:], in1=tc1[:], op=ALU.mult)
            nc.sync.dma_start(c1s_d[:], c1s[:])
            nc.sync.dma_start(h1s_d[:], h1s[:])

            # ---------------- AllGather h1 -------------------------------
            h1b_in = dpool.tile([HS, B], mdt)
            nc.gpsimd.dma_start(h1b_in[:], h1s[:])  # cast f32->mdt if bf16
            h1b_out = dpool.tile([H, B], mdt, addr_space="Shared")
            nc.gpsimd.collective_compute(
                "AllGather", ALU.bypass,
                replica_groups=[list(range(NCORE))],
                ins=[h1b_in.opt()], outs=[h1b_out.opt()],
            )
            h1T = apool.tile([P, KT, B], mdt)
            nc.sync.dma_start(h1T[:], wstripe(h1b_out[:]))

            # ---------------- caption cell: h2h terms early --------------
            h20T = apool.tile([P, KT, B], mdt)
            load(h20T[:], wstripe(h20T_d[:]))
            capb = cpool.tile([P, 4], F32)
            nc.sync.dma_start(capb[:], capb_d[:].rearrange("(g p) -> p g", p=P))
            c20Ts = apool.tile([P, B], F32)
            nc.sync.dma_start(c20Ts[:], c20Ts_d[:])

            cap_ps = []
            # caption gate order: i, g, o, f  (t2h feeds o and f)
            for gt in range(4):
                wh2 = wpool.tile([P, KT, P], mdt, tag="w", name="wh2h")
                load(wh2[:], wstripe(h2hT_d[:, gt * P : (gt + 1) * P]))
                pc = psum_t(B, name="ps_cap")
                for kt in range(KT):
                    nc.tensor.matmul(
                        pc, lhsT=wh2[:, kt, :], rhs=h20T[:, kt, :],
                        start=(kt == 0), stop=False,
                    )
                if gt >= 2:
                    wt2 = wpool.tile([P, KT, P], mdt, tag="w", name="wt2h")
                    load(wt2[:], wstripe(t2hT_d[:, (gt - 2) * P : (gt - 1) * P]))
                    for kt in range(KT):
                        nc.tensor.matmul(
                            pc, lhsT=wt2[:, kt, :], rhs=h1T[:, kt, :],
                            start=False, stop=False,
                        )
                cap_ps.append(pc)

            # ---------------- attention: WgHC = wg@h1c + wc@c2c ----------
            # h1Tc[h, b'] = h1T[h, core_batch_slice]: the AG result viewed as
            # [H*NCORE, BS] rows; per-core row offsets select this core's
            # batch-group column block (SPMD-safe: offsets come in as data).
            h1sel = cpool.tile([P, KT], U32)
            nc.sync.dma_start(h1sel[:], h1sel_d[:].rearrange("(kt p) -> p kt", p=P))
            h1g_view = h1b_out[:].rearrange("h (g b) -> (h g) b", b=BS)
            h1Tc = apool.tile([P, KT, BS], mdt)
            for kt in range(KT):
                nc.gpsimd.indirect_dma_start(
                    out=h1Tc[:, kt, :],
                    out_offset=None,
                    in_=h1g_view,
                    in_offset=bass.IndirectOffsetOnAxis(ap=h1sel[:, kt : kt + 1], axis=0),
                )
            c20Tb = apool.tile([P, KT, BS], mdt)
            load(c20Tb[:], wstripe(c20Tb_d[:]))
            attb = cpool.tile([P, KT], F32)
            nc.sync.dma_start(attb[:], attb_d[:].rearrange("(jt p) -> p jt", p=P))
            wghc = apool.tile([P, KT, BS], F32)
            for jt in range(KT):
                wg_t = wpool.tile([P, KT, P], mdt, tag="w", name="wg")
                load(wg_t[:], wstripe(wgT_d[:, jt * P : (jt + 1) * P]))
                wc_t = wpool.tile([P, KT, P], mdt, tag="w", name="wc")
                load(wc_t[:], wstripe(wcT_d[:, jt * P : (jt + 1) * P]))
                pw = psum_t(BS, name="ps_wghc")
                for kt in range(KT):
                    nc.tensor.matmul(
                        pw, lhsT=wg_t[:, kt, :], rhs=h1Tc[:, kt, :],
                        start=(kt == 0), stop=False,
                    )
                for kt in range(KT):
                    nc.tensor.matmul(
                        pw, lhsT=wc_t[:, kt, :], rhs=c20Tb[:, kt, :],
                        start=False, stop=(kt == KT - 1),
                    )
                nc.scalar.activation(
                    wghc[:, jt, :], pw, AF.Identity, bias=attb[:, jt : jt + 1]
                )

            # ---------------- attention: tanh(Wv_V + WgHC) ---------------
            vT = apool.tile([P, KT, BN], mdt)
            load(vT[:], wstripe(vT_d[:]))
            tanhed = apool.tile([P, KT, BN], mdt)
            for jt in range(KT):
                wv_t = wpool.tile([P, KT, P], mdt, tag="w", name="wv")
                load(wv_t[:], wstripe(wvT_d[:, jt * P : (jt + 1) * P]))
                for ch in range(NCH):
                    pv = psum_t(BNCH, name="ps_wvv")
                    for kt in range(KT):
                        nc.tensor.matmul(
                            pv,
                            lhsT=wv_t[:, kt, :],
                            rhs=vT[:, kt, ch * BNCH : (ch + 1) * BNCH],
                            start=(kt == 0), stop=(kt == KT - 1),
                        )
                    tt = tpool.tile([P, BNCH], F32, tag="tt", name="tt")
                    nc.vector.tensor_tensor(
                        out=tt[:].rearrange("p (b n) -> p b n", n=N),
                        in0=pv.rearrange("p (b n) -> p b n", n=N),
                        in1=wghc[:, jt, ch * BCH : (ch + 1) * BCH, None].to_broadcast(
                            [P, BCH, N]
                        ),
                        op=ALU.add,
                    )
                    nc.scalar.activation(
                        tanhed[:, jt, ch * BNCH : (ch + 1) * BNCH], tt[:], AF.Tanh
                    )

            # ---------------- attention: alpha, softmax, alpha_v ---------
            whT = cpool.tile([P, KT], mdt)
            load(whT[:], whT_d[:].rearrange("(kt p) -> p kt", p=P))
            asb = apool.tile([1, BN], F32)
            for ch in range(NCH):
                pa = psum_t(BNCH, parts=1, name="ps_alpha")
                for kt in range(KT):
                    nc.tensor.matmul(
                        pa,
                        lhsT=whT[:, kt : kt + 1],
                        rhs=tanhed[:, kt, ch * BNCH : (ch + 1) * BNCH],
                        start=(kt == 0), stop=(kt == KT - 1),
                    )
                nc.scalar.activation(asb[:, ch * BNCH : (ch + 1) * BNCH], pa, AF.Identity)

            asb3 = asb[:].rearrange("p (b n) -> p b n", n=N)
            amax = cpool.tile([1, BS], F32)
            nc.vector.reduce_max(amax[:], asb3, axis=AX.X)
            nc.vector.tensor_tensor(
                out=asb3, in0=asb3,
                in1=amax[:, :, None].to_broadcast([1, BS, N]),
                op=ALU.subtract,
            )
            nc.scalar.activation(asb[:], asb[:], AF.Exp)
            asum = cpool.tile([1, BS], F32)
            nc.vector.reduce_sum(asum[:], asb3, axis=AX.X)
            arec = cpool.tile([1, BS], F32)
            nc.vector.reciprocal(arec[:], asum[:])
            att = cpool.tile([1, BN], mdt)
            nc.vector.tensor_tensor(
                out=att[:].rearrange("p (b n) -> p b n", n=N),
                in0=asb3,
                in1=arec[:, :, None].to_broadcast([1, BS, N]),
                op=ALU.mult,
            )

            # broadcast att across partitions via K=1 outer product
            ones1 = cpool.tile([1, P], mdt)
            nc.gpsimd.memset(ones1[:], 1.0)
            attR = apool.tile([P, BN], mdt)
            for ch in range(NCH):
                pb = psum_t(BNCH, name="ps_attR")
                nc.tensor.matmul(
                    pb, lhsT=ones1[:], rhs=att[:, ch * BNCH : (ch + 1) * BNCH],
                    start=True, stop=True,
                )
                nc.vector.tensor_copy(attR[:, ch * BNCH : (ch + 1) * BNCH], pb)

            # alpha_vT[h, b] = sum_n vT[h, (b,n)] * attR[(b,n)]
            avT = apool.tile([P, KT, BS], F32)
            for kt in range(KT):
                sc = tpool.tile([P, BN], F32, tag="sc", name="sc")
                nc.vector.tensor_tensor(
                    out=sc[:], in0=vT[:, kt, :], in1=attR[:], op=ALU.mult
                )
                nc.vector.reduce_sum(
                    avT[:, kt, :], sc[:].rearrange("p (b n) -> p b n", n=N), axis=AX.X
                )

            # ---------------- AllGather alpha_v --------------------------
            avb_in = dpool.tile([H, BS], mdt)
            nc.gpsimd.dma_start(
                avb_in[:].rearrange("(kt p) b -> p kt b", p=P), avT[:]
            )
            avb_out = dpool.tile([NCORE * H, BS], mdt, addr_space="Shared")
            nc.gpsimd.collective_compute(
                "AllGather", ALU.bypass,
                replica_groups=[list(range(NCORE))],
                ins=[avb_in.opt()], outs=[avb_out.opt()],
            )
            av = apool.tile([P, KT, NCORE, BS], mdt)
            for r in range(NCORE):
                nc.sync.dma_start(
                    av[:, :, r, :], wstripe(avb_out[r * H : (r + 1) * H, :])
                )

            # ---------------- caption cell: i2h terms + gates ------------
            for gt in range(4):
                wi2 = wpool.tile([P, KT, P], mdt, tag="w", name="wi2h")
                load(wi2[:], wstripe(i2hT_d[:, gt * P : (gt + 1) * P]))
                pc = cap_ps[gt]
                for kt in range(KT):
                    nc.tensor.matmul(
                        pc, lhsT=wi2[:, kt, :], rhs=av[:, kt, :, :],
                        start=False, stop=(kt == KT - 1),
                    )
            cga = []
            for gt in range(4):
                fn = AF.Tanh if gt == 1 else AF.Sigmoid
                cg = apool.tile([P, B], F32, name=f"cga{gt}")
                nc.scalar.activation(cg[:], cap_ps[gt], fn, bias=capb[:, gt : gt + 1])
                cga.append(cg)

            c2s = apool.tile([P, B], F32)
            tmp_gi = tpool.tile([P, B], F32, tag="ew", name="tmp_gi")
            nc.vector.tensor_tensor(out=tmp_gi[:], in0=cga[1][:], in1=cga[0][:], op=ALU.mult)
            tmp_cf = tpool.tile([P, B], F32, tag="ew", name="tmp_cf")
            nc.vector.tensor_tensor(out=tmp_cf[:], in0=c20Ts[:], in1=cga[3][:], op=ALU.mult)
            nc.vector.tensor_tensor(out=c2s[:], in0=tmp_gi[:], in1=tmp_cf[:], op=ALU.add)
            tc2 = tpool.tile([P, B], F32, tag="ew", name="tc2")
            nc.scalar.activation(tc2[:], c2s[:], AF.Tanh)
            h2s = apool.tile([P, B], F32)
            nc.vector.tensor_tensor(out=h2s[:], in0=tc2[:], in1=cga[2][:], op=ALU.mult)
            nc.sync.dma_start(c2s_d[:], c2s[:])
            nc.sync.dma_start(h2s_d[:], h2s[:])

            # ---------------- AllGather h2 -------------------------------
            h2b_in = dpool.tile([HS, B], mdt)
            nc.gpsimd.dma_start(h2b_in[:], h2s[:])
            h2b_out = dpool.tile([H, B], mdt, addr_space="Shared")
            nc.gpsimd.collective_compute(
                "AllGather", ALU.bypass,
                replica_groups=[list(range(NCORE))],
                ins=[h2b_in.opt()], outs=[h2b_out.opt()],
            )
            h2T = apool.tile([P, KT, B], mdt)
            nc.sync.dma_start(h2T[:], wstripe(h2b_out[:]))

            # ---------------- MLP vocab projection -----------------------
            mlpb = cpool.tile([1, VSP], mdt)
            load(mlpb[:], mlpb_d[None, :])
            for ch in range(N_MLP_CH):
                wm = mpool.tile([P, KT, MLP_CH], mdt, tag="wm", name="wm")
                load(wm[:], wstripe(mlpT_d[:, ch * MLP_CH : (ch + 1) * MLP_CH]))
                for bt in range(2):
                    pm = psum_t(MLP_CH, name="ps_mlp")
                    nc.tensor.matmul(
                        pm, lhsT=ones1[:],
                        rhs=mlpb[:, ch * MLP_CH : (ch + 1) * MLP_CH],
                        start=True, stop=False,
                    )
                    for kt in range(KT):
                        nc.tensor.matmul(
                            pm,
                            lhsT=h2T[:, kt, bt * P : (bt + 1) * P],
                            rhs=wm[:, kt, :],
                            start=False, stop=(kt == KT - 1),
                        )
                    ot = opool.tile([P, MLP_CH], F32, tag="ot", name="ot")
                    nc.vector.tensor_copy(ot[:], pm)
                    nc.sync.dma_start(
                        out_d[bt * P : (bt + 1) * P, ch * MLP_CH : (ch + 1) * MLP_CH],
                        ot[:],
                    )
    nc.finalize()
    return nc


def _prep_inputs(inputs, mode):
    hdt = _host_dt(mode)
    f32 = np.float32

    def cvt(x):
        return np.ascontiguousarray(x.astype(hdt))

    v = np.asarray(inputs["v"], f32)
    h1_0 = np.asarray(inputs["h1_0"], f32)
    c1_0 = np.asarray(inputs["c1_0"], f32)
    h2_0 = np.asarray(inputs["h2_0"], f32)
    c2_0 = np.asarray(inputs["c2_0"], f32)
    ids = np.asarray(inputs["input_ids"]).astype(np.uint32)
    embed_W = np.asarray(inputs["embed_W"], f32)

    lstm_Wih = np.asarray(inputs["lstm_Wih"], f32)
    lstm_Whh = np.asarray(inputs["lstm_Whh"], f32)
    lstm_b = np.asarray(inputs["lstm_bih"], f32) + np.asarray(inputs["lstm_bhh"], f32)
    wv_W = np.asarray(inputs["wv_W"], f32)
    wg_W = np.asarray(inputs["wg_W"], f32)
    wc_W = np.asarray(inputs["wc_W"], f32)
    attb = (
        np.asarray(inputs["wv_b"], f32)
        + np.asarray(inputs["wg_b"], f32)
        + np.asarray(inputs["wc_b"], f32)
    )
    wh_W = np.asarray(inputs["wh_W"], f32)
    i2h_W = np.asarray(inputs["i2h_W"], f32)
    i2h_b = np.asarray(inputs["i2h_b"], f32)
    h2h_W = np.asarray(inputs["h2h_W"], f32)
    h2h_b = np.asarray(inputs["h2h_b"], f32)
    t2h_W = np.asarray(inputs["t2h_W"], f32)
    t2h_b = np.asarray(inputs["t2h_b"], f32)
    mlp_W = np.asarray(inputs["mlp_W"], f32)
    mlp_b = np.asarray(inputs["mlp_b"], f32)

    embed_c = cvt(embed_W)
    h10T = cvt(h1_0.T)
    h20T = cvt(h2_0.T)
    c10T_f = np.ascontiguousarray(c1_0.T)        # f32
    c20T_f = np.ascontiguousarray(c2_0.T)        # f32
    whT = cvt(wh_W[0])
    wvT = cvt(wv_W.T)
    wgT = cvt(wg_W.T)
    wcT = cvt(wc_W.T)

    def col_slice(Wrows, ngates, c):
        # pick this core's HS rows of each gate, transpose -> [K, ngates*HS]
        rows = np.concatenate(
            [Wrows[g * H + c * HS : g * H + (c + 1) * HS] for g in range(ngates)], 0
        )
        return cvt(rows.T)

    def b_slice(bias, ngates, c):
        return np.concatenate(
            [bias[g * H + c * HS : g * H + (c + 1) * HS] for g in range(ngates)], 0
        ).astype(f32)

    ib_hb = i2h_b + h2h_b
    in_maps = []
    for c in range(NCORE):
        bs = slice(c * BS, (c + 1) * BS)
        hs = slice(c * HS, (c + 1) * HS)
        capb = np.concatenate(
            [
                ib_hb[0 * H + c * HS : 0 * H + (c + 1) * HS],
                ib_hb[1 * H + c * HS : 1 * H + (c + 1) * HS],
                ib_hb[2 * H + c * HS : 2 * H + (c + 1) * HS] + t2h_b[0 * H + c * HS : 0 * H + (c + 1) * HS],
                ib_hb[3 * H + c * HS : 3 * H + (c + 1) * HS] + t2h_b[1 * H + c * HS : 1 * H + (c + 1) * HS],
            ],
            0,
        ).astype(f32)
        mlpT = np.zeros((H, VSP), np.float32)
        mlpT[:, :VS] = mlp_W[c * VS : (c + 1) * VS].T
        mlpb = np.zeros((VSP,), np.float32)
        mlpb[:VS] = mlp_b[c * VS : (c + 1) * VS]
        in_maps.append(
            {
                "ids": ids,
                "h1sel": (np.arange(H, dtype=np.uint32) * NCORE + c).astype(np.uint32),
                "embed": embed_c,
                "wihT": col_slice(lstm_Wih, 4, c),
                "whhT": col_slice(lstm_Whh, 4, c),
                "lstmb": b_slice(lstm_b, 4, c),
                "h10T": h10T,
                "c10Ts": np.ascontiguousarray(c10T_f[hs]),
                "vT": cvt(v[bs].reshape(BN, H).T),
                "wvT": wvT,
                "wgT": wgT,
                "wcT": wcT,
                "attb": attb,
                "whT": whT,
                "c20Tb": cvt(c2_0[bs].T),
                "i2hT": col_slice(i2h_W, 4, c),
                "h2hT": col_slice(h2h_W, 4, c),
                "t2hT": col_slice(t2h_W, 2, c),
                "capb": capb,
                "h20T": h20T,
                "c20Ts": np.ascontiguousarray(c20T_f[hs]),
                "mlpT": cvt(mlpT),
                "mlpb": cvt(mlpb),
            }
        )
    return in_maps


_NC_CACHE = {}


def _get_nc(mode):
    if mode not in _NC_CACHE:
        _NC_CACHE[mode] = build_kernel(mode)
    return _NC_CACHE[mode]


def run(inputs, mode=MM_MODE, **run_kwargs):
    in_maps = _prep_inputs(inputs, mode)
    nc = _get_nc(mode)
    res = run_bass_kernel_spmd(nc, in_maps, core_ids=list(range(NCORE)), **run_kwargs)
    outs = res.results
    logits = np.concatenate([outs[c]["logits"][:, :VS] for c in range(NCORE)], 1)
    h1 = np.concatenate([outs[c]["h1s"] for c in range(NCORE)], 0).T
    c1 = np.concatenate([outs[c]["c1s"] for c in range(NCORE)], 0).T
    h2 = np.concatenate([outs[c]["h2s"] for c in range(NCORE)], 0).T
    c2 = np.concatenate([outs[c]["c2s"] for c in range(NCORE)], 0).T
    result = tuple(
        np.ascontiguousarray(x, np.float32) for x in (logits, h1, c1, h2, c2)
    )
    return result, res


def kernel(**inputs):
    result, _ = run(inputs)
    return result


def bench(inputs, mode=MM_MODE, iters=20):
    """Repeat-execute the sharded program with device-resident inputs and
    return (per-call wall times, outputs-of-last-call as core dicts)."""
    import time
    import jax
    from jax.sharding import Mesh, PartitionSpec, NamedSharding
    from jax.experimental.shard_map import shard_map
    from concourse import bass2jax

    in_maps = _prep_inputs(inputs, mode)
    nc = _get_nc(mode)
    bass2jax.install_neuronx_cc_hook()

    partition_name = nc.partition_id_tensor.name if nc.partition_id_tensor else None
    in_names, out_names, out_avals, zero_outs = [], [], [], []
    for alloc in nc.m.functions[0].allocations:
        if not isinstance(alloc, mybir.MemoryLocationSet):
            continue
        name = alloc.memorylocations[0].name
        if alloc.kind == "ExternalInput":
            if name != partition_name:
                in_names.append(name)
        elif alloc.kind == "ExternalOutput":
            shape = tuple(alloc.tensor_shape)
            dtype = mybir.dt.np(alloc.dtype)
            out_names.append(name)
            out_avals.append(jax.core.ShapedArray(shape, dtype))
            zero_outs.append(np.zeros(shape, dtype))
    n_params = len(in_names)
    in_names_all = in_names + out_names + ([partition_name] if partition_name else [])

    def _body(*args):
        operands = list(args)
        if partition_name:
            operands.append(bass2jax.partition_id_tensor())
        return tuple(
            bass2jax._bass_exec_p.bind(
                *operands,
                out_avals=tuple(out_avals),
                in_names=tuple(in_names_all),
                out_names=tuple(out_names),
                lowering_input_output_aliases=(),
                sim_require_finite=True,
                sim_require_nnan=True,
                nc=nc,
            )
        )

    devices = jax.devices()[:NCORE]
    mesh = Mesh(np.asarray(devices), ("core",))
    sharded = jax.jit(
        shard_map(
            _body,
            mesh=mesh,
            in_specs=(PartitionSpec("core"),) * (n_params + len(out_names)),
            out_specs=(PartitionSpec("core"),) * len(out_names),
            check_rep=False,
        ),
        keep_unused=True,
    )
    concat_in = [
        np.concatenate([np.asarray(m[k]) for m in in_maps], axis=0) for k in in_names
    ]
    concat_zeros = [
        np.zeros((NCORE * z.shape[0], *z.shape[1:]), z.dtype) for z in zero_outs
    ]
    sh = NamedSharding(mesh, PartitionSpec("core"))
    dev_in = [jax.device_put(x, sh) for x in concat_in + concat_zeros]

    out = sharded(*dev_in)
    jax.block_until_ready(out)
    times = []
    for _ in range(iters):
        t0 = time.perf_counter()
        out = sharded(*dev_in)
        jax.block_until_ready(out)
        times.append(time.perf_counter() - t0)

    results = [
        {
            name: np.asarray(out[i]).reshape(NCORE, *out_avals[i].shape)[c]
            for i, name in enumerate(out_names)
        }
        for c in range(NCORE)
    ]
    return times, results
